# revision 16
# baseline (speedup 1.0000x reference)
"""Trainium2 Bass kernel for AnemoneSparseMoeBlock (top-2 of 8 experts MoE).

Expert parallelism across 8 NeuronCores: core e holds expert e's weights
(host-side layout transforms only). The router runs replicated per token
shard in f32; combine weights are AllGathered. Each core compacts the token
ids routed to its expert with an on-device prefix-sum (scan + triangular
matmul + one-hot matmuls), gathers just those token rows (bf16) via
indirect DMA, runs the expert MLP in bf16 with f32 PSUM accumulation,
scales by the top-2 softmax weight, scatters into a zeroed [T, D] bf16
buffer, and the 8 cores ReduceScatter-combine in two token-block chunks so
the first RS overlaps the second block's compute.

kernel(**inputs) takes FULL inputs, returns (out [B,S,D], router_logits).
"""

import numpy as np

import concourse.bass as bass
import concourse.mybir as mybir
import concourse.tile as tile
from concourse import bacc
from concourse.bass_utils import run_bass_kernel_spmd
from concourse.masks import make_identity

P = 128
NCORES = 8

F32 = mybir.dt.float32
BF16 = mybir.dt.bfloat16
I32 = mybir.dt.int32
AF = mybir.ActivationFunctionType
OP = mybir.AluOpType


class Cfg:
    def __init__(self, D=1024, F=4096, E=8, T=8192, NBLK=2, NS=10, NSG=4):
        self.D, self.F, self.E, self.T = D, F, E, T
        self.DK = D // P          # D chunks of 128
        self.NFC = F // P         # F chunks of 128
        self.NBLK = NBLK          # token blocks
        self.TBLK = T // NBLK     # tokens per block
        self.CC = self.TBLK // P  # free cols per partition per block
        self.NS = NS              # slot chunks (128) per block -> capacity
        self.CAP = NS * P
        self.NSG = NSG            # weight super-groups per block
        self.FCG = self.NFC // NSG  # f-chunks per super group
        self.TPC = T // NCORES    # tokens per core shard (router)
        self.XROWS = T + P        # x padded with a zero row block
        self.TT2 = min(256, self.CAP)      # mm1 token tile
        self.NTT2 = self.CAP // self.TT2   # mm1 tiles per block
        self.JT = self.TT2 // P            # 128-slot chunks per mm1 tile
        self.NH = max(1, self.D // 512)    # mm2 N splits (<=1 psum bank each)
        self.DCH = self.DK // self.NH      # d-chunks per mm2 matmul


FULL = Cfg()


def build(cfg: Cfg) -> bass.Bass:
    c = cfg
    nc = bacc.Bacc(None, target_bir_lowering=False, num_devices=NCORES)

    # ---- I/O ----
    x = nc.dram_tensor("x", [c.XROWS, c.D], F32, kind="ExternalInput")
    xts = nc.dram_tensor("xts", [c.D, c.TPC], F32, kind="ExternalInput")
    rwt = nc.dram_tensor("rwt", [c.D, c.E], F32, kind="ExternalInput")
    gsw = nc.dram_tensor("gsw", [c.NFC, P, c.DK, P], F32, kind="ExternalInput")
    usw = nc.dram_tensor("usw", [c.NFC, P, c.DK, P], F32, kind="ExternalInput")
    dsw = nc.dram_tensor("dsw", [c.NFC, P, c.DK, P], F32, kind="ExternalInput")
    eoh = nc.dram_tensor("eoh", [P, c.E], F32, kind="ExternalInput")

    out = nc.dram_tensor("out", [c.TPC, c.D], F32, kind="ExternalOutput")
    logits_o = nc.dram_tensor("logits", [c.TPC, c.E], F32, kind="ExternalOutput")
    debug = getattr(c, "debug", False)
    if debug:
        dbg_gidx = nc.dram_tensor("dbg_gidx", [c.NBLK, P, c.NS], I32,
                                  kind="ExternalOutput")
        dbg_sidx = nc.dram_tensor("dbg_sidx", [c.NBLK, P, c.NS], I32,
                                  kind="ExternalOutput")
        dbg_wslot = nc.dram_tensor("dbg_wslot", [c.NBLK, P, c.NS], F32,
                                   kind="ExternalOutput")
        dbg_od = nc.dram_tensor("dbg_od", [c.T, c.D], BF16,
                                kind="ExternalOutput")
        dbg_xtg = nc.dram_tensor("dbg_xtg", [c.NBLK, P, c.DK, c.CAP], BF16,
                                 kind="ExternalOutput")
        dbg_pm = nc.dram_tensor("dbg_pm", [c.NBLK, P, c.CC], F32,
                                kind="ExternalOutput")
        dbg_idw = nc.dram_tensor("dbg_idw", [c.NBLK, P, c.CC, 4], F32,
                                 kind="ExternalOutput")
        dbg_glf = nc.dram_tensor("dbg_glf", [c.NBLK, P, c.NS, 4], F32,
                                 kind="ExternalOutput")

    rg = [list(range(NCORES))]

    with tile.TileContext(nc) as tc:
        with (
            tc.tile_pool(name="const", bufs=1) as constp,
            tc.tile_pool(name="wpool", bufs=c.FCG) as wpool,
            tc.tile_pool(name="xtg", bufs=getattr(c, "xtg_bufs", 2)) as xtgp,
            tc.tile_pool(name="xg", bufs=2) as xgp,
            tc.tile_pool(name="outsb", bufs=2 * c.NTT2) as outsbp,
            tc.tile_pool(name="stg", bufs=2) as stgp,
            tc.tile_pool(name="rtr", bufs=1) as rtrp,
            tc.tile_pool(name="cmp", bufs=2) as cmpp,
            tc.tile_pool(name="spool", bufs=3) as spool,
            tc.tile_pool(name="gu", bufs=2, space="PSUM") as gups,
            tc.tile_pool(name="po", bufs=2, space="PSUM") as pops,
            tc.tile_pool(name="tp", bufs=1, space="PSUM") as tpps,
            tc.tile_pool(name="aux", bufs=1, space="PSUM") as auxps,
            tc.tile_pool(name="dram", bufs=1, space="DRAM") as dr,
        ):
            # ---------- constants ----------
            ident = constp.tile([P, P], BF16, tag="ident")
            make_identity(nc, ident[:])

            it_i = spool.tile([P, c.CAP], I32, tag="S")  # transient slot reuse
            nc.gpsimd.iota(it_i[:], pattern=[[1, c.CAP]], base=0,
                           channel_multiplier=0)
            slotio = constp.tile([P, c.CAP], F32, tag="slotio")
            nc.vector.tensor_copy(slotio[:], it_i[:])

            iof_i = cmpp.tile([P, P], I32, tag="iof")
            nc.gpsimd.iota(iof_i[:], pattern=[[1, P]], base=0,
                           channel_multiplier=0)
            iop_i = cmpp.tile([P, 1], I32, tag="iop")
            nc.gpsimd.iota(iop_i[:], pattern=[[0, 1]], base=0,
                           channel_multiplier=1)
            iof = cmpp.tile([P, P], F32, tag="ioff")
            nc.vector.tensor_copy(iof[:], iof_i[:])
            iop = cmpp.tile([P, 1], F32, tag="iopf")
            nc.vector.tensor_copy(iop[:], iop_i[:])
            L128 = constp.tile([P, P], F32, tag="L128")
            # L[k, m] = 1 if m > k  (strict lower triangular as [K, M])
            nc.vector.tensor_scalar(L128[:], iof[:], iop[:, 0:1], None,
                                    op0=OP.is_gt)

            eoh_sb = constp.tile([P, c.E], F32, tag="eoh")
            nc.sync.dma_start(eoh_sb[:], eoh[:])

            zero16 = constp.tile([P, c.D], BF16, tag="zero16")
            nc.vector.memset(zero16[:], 0.0)

            rwt_sb = constp.tile([P, c.DK, c.E], F32, tag="rwt")
            nc.sync.dma_start(rwt_sb[:],
                              rwt[:].rearrange("(k p) e -> p k e", p=P))

            # ---------- internal DRAM ----------
            x16 = dr.tile([c.XROWS, c.D], BF16, name="x16")
            g16 = dr.tile([c.NFC, P, c.DK, P], BF16, name="g16")
            u16 = dr.tile([c.NFC, P, c.DK, P], BF16, name="u16")
            d16 = dr.tile([c.NFC, P, c.DK, P], BF16, name="d16")
            cwsh = dr.tile([c.TPC, c.E], F32, name="cwsh")
            cw_all = dr.tile([c.T, c.E], F32, addr_space="Shared", name="cw_all")
            out_dram = dr.tile([c.T, c.D], BF16, name="out_dram")
            rs_o = [dr.tile([c.TPC // c.NBLK, c.D], BF16, name=f"rs_o_{h}")
                    for h in range(c.NBLK)]

            # ---------- cast passes (SWDGE dtype-cast DMAs) ----------
            for i in range(2):
                lo = i * (c.XROWS // 2)
                hi = c.XROWS if i == 1 else c.XROWS // 2
                nc.gpsimd.dma_start(x16[lo:hi, :], x[lo:hi, :])
            for sg in range(c.NSG):
                lo, hi = sg * c.FCG, (sg + 1) * c.FCG
                nc.gpsimd.dma_start(g16[lo:hi], gsw[lo:hi])
                nc.gpsimd.dma_start(u16[lo:hi], usw[lo:hi])
                nc.gpsimd.dma_start(d16[lo:hi], dsw[lo:hi])

            # ---------- zero the combine buffer ----------
            for j in range(c.T // P):
                nc.sync.dma_start(out_dram[j * P:(j + 1) * P, :], zero16[:])

            # ---------- router (f32) over this core's token shard ----------
            n_rtt = c.TPC // P
            QT = 1
            for q in range(n_rtt // QT):
                xtsh = rtrp.tile([P, c.DK, QT * P], F32, tag="xtsh")
                nc.sync.dma_start(
                    xtsh[:],
                    xts[:, q * QT * P:(q + 1) * QT * P].rearrange(
                        "(k p) t -> p k t", p=P),
                )
                for j in range(QT):
                    tt = q * QT + j
                    pl = auxps.tile([P, 128], F32, tag="aux")
                    for k in range(c.DK):
                        nc.tensor.matmul(
                            pl[:, :c.E], lhsT=xtsh[:, k, j * P:(j + 1) * P],
                            rhs=rwt_sb[:, k, :],
                            start=(k == 0), stop=(k == c.DK - 1))
                    lsb = cmpp.tile([P, c.E], F32, tag="lsb")
                    nc.vector.tensor_copy(lsb[:], pl[:, :c.E])
                    nc.sync.dma_start(logits_o[tt * P:(tt + 1) * P, :], lsb[:])
                    # softmax (f32)
                    mx = cmpp.tile([P, 1], F32, tag="mx")
                    nc.vector.reduce_max(mx[:], lsb[:], axis=mybir.AxisListType.X)
                    nmx = cmpp.tile([P, 1], F32, tag="nmx")
                    nc.vector.tensor_scalar_mul(nmx[:], mx[:], -1.0)
                    ex = cmpp.tile([P, c.E], F32, tag="ex")
                    nc.scalar.activation(ex[:], lsb[:], AF.Exp, bias=nmx[:, 0:1])
                    sm = cmpp.tile([P, 1], F32, tag="sm")
                    nc.vector.reduce_sum(sm[:], ex[:], axis=mybir.AxisListType.X)
                    rc = cmpp.tile([P, 1], F32, tag="rc")
                    nc.vector.reciprocal(rc[:], sm[:])
                    # top-2 mask from logits (same order as probs)
                    m8 = cmpp.tile([P, 8], F32, tag="m8")
                    nc.vector.max(out=m8[:], in_=lsb[:])
                    msk = cmpp.tile([P, c.E], F32, tag="msk")
                    nc.vector.tensor_scalar(
                        msk[:], lsb[:], m8[:, 1:2], None, op0=OP.is_ge)
                    cwt = cmpp.tile([P, c.E], F32, tag="cwt")
                    nc.vector.tensor_scalar_mul(cwt[:], ex[:], rc[:, 0:1])
                    nc.vector.tensor_mul(cwt[:], cwt[:], msk[:])
                    nc.sync.dma_start(cwsh[tt * P:(tt + 1) * P, :], cwt[:])

            nc.gpsimd.collective_compute(
                "AllGather", OP.bypass, replica_groups=rg,
                ins=[cwsh[:].opt()], outs=[cw_all[:].opt()],
            )

            # ---------- per token-block ----------
            for h in range(c.NBLK):
                # --- combine weights for my expert, this block ---
                cwb = cmpp.tile([P, c.CC, c.E], F32, tag="cwb")
                nc.sync.dma_start(
                    cwb[:],
                    cw_all[h * c.TBLK:(h + 1) * c.TBLK, :].rearrange(
                        "(p cc) e -> p cc e", cc=c.CC),
                )
                wex = cmpp.tile([P, c.CC, c.E], F32, tag="wex")
                nc.vector.tensor_mul(
                    wex[:], cwb[:],
                    eoh_sb[:, None, :].to_broadcast([P, c.CC, c.E]))
                we = cmpp.tile([P, c.CC], F32, tag="we")
                nc.vector.reduce_sum(we[:], wex[:], axis=mybir.AxisListType.X)

                # --- prefix-sum compaction ---
                msk2 = cmpp.tile([P, c.CC], F32, tag="msk2")
                nc.vector.tensor_scalar(msk2[:], we[:], 0.0, None, op0=OP.is_gt)
                inc = cmpp.tile([P, c.CC], F32, tag="inc")
                nc.vector.tensor_tensor_scan(
                    inc[:], msk2[:], msk2[:], 0.0, op0=OP.add, op1=OP.bypass)
                excl = cmpp.tile([P, c.CC], F32, tag="excl")
                nc.vector.tensor_sub(excl[:], inc[:], msk2[:])
                pcx = auxps.tile([P, 128], F32, tag="aux")
                nc.tensor.matmul(pcx[:, :1], lhsT=L128[:],
                                 rhs=inc[:, c.CC - 1:c.CC],
                                 start=True, stop=True)
                crossx = cmpp.tile([P, 1], F32, tag="crossx")
                nc.vector.tensor_copy(crossx[:], pcx[:, :1])
                pm = cmpp.tile([P, c.CC], F32, tag="pm")
                nc.vector.tensor_scalar_add(pm[:], excl[:], crossx[:, 0:1])
                nc.vector.tensor_scalar(pm[:], pm[:], 1.0, None, op0=OP.add)
                nc.vector.tensor_mul(pm[:], pm[:], msk2[:])
                nc.vector.tensor_scalar(pm[:], pm[:], -1.0, None, op0=OP.add)

                # idw rhs columns: [token_id, 1, weight, 0]
                ids_i = cmpp.tile([P, c.CC], I32, tag="ids_i")
                nc.gpsimd.iota(ids_i[:], pattern=[[1, c.CC]], base=h * c.TBLK,
                               channel_multiplier=c.CC)
                idw = cmpp.tile([P, c.CC, 4], F32, tag="idw")
                nc.vector.tensor_copy(idw[:, :, 0], ids_i[:])
                nc.vector.memset(idw[:, :, 1], 1.0)
                nc.vector.tensor_copy(idw[:, :, 2], we[:])
                nc.vector.memset(idw[:, :, 3], 0.0)

                # NOTE: accumulation chains into slices of one PSUM bank must
                # be contiguous (interleaved chains corrupt the bank), so we
                # run a full chain per slot-chunk over a resident group of S
                # tiles and accumulate groups in SBUF.
                GS = min(2, c.CC)
                glf = cmpp.tile([P, c.NS, 4], F32, tag="glf")
                for ccg in range(c.CC // GS):
                    stiles = []
                    for ci in range(GS):
                        cc = ccg * GS + ci
                        S = spool.tile([P, c.CAP], F32, tag="S", name=f"S{cc}")
                        nc.vector.tensor_scalar(
                            S[:], slotio[:], pm[:, cc:cc + 1], None,
                            op0=OP.is_equal)
                        stiles.append(S)
                    for s in range(c.NS):
                        pgl = auxps.tile([P, 4], F32, tag="aux")
                        for ci in range(GS):
                            nc.tensor.matmul(
                                pgl[:], lhsT=stiles[ci][:, s * P:(s + 1) * P],
                                rhs=idw[:, ccg * GS + ci, :],
                                start=(ci == 0), stop=(ci == GS - 1))
                        if ccg == 0:
                            nc.vector.tensor_copy(glf[:, s, :], pgl[:])
                        else:
                            nc.vector.tensor_add(glf[:, s, :], glf[:, s, :],
                                                 pgl[:])
                # gather idx: id + (1-valid)*T ; scatter idx: id + (1-valid)*1e6
                gadj = cmpp.tile([P, c.NS], F32, tag="gadj")
                nc.vector.tensor_scalar(
                    gadj[:], glf[:, :, 1], -float(c.T), float(c.T),
                    op0=OP.mult, op1=OP.add)
                gidx_f = cmpp.tile([P, c.NS], F32, tag="gidx_f")
                nc.vector.tensor_add(gidx_f[:], gadj[:], glf[:, :, 0])
                gidx = cmpp.tile([P, c.NS], I32, tag="gidx")
                nc.vector.tensor_copy(gidx[:], gidx_f[:])
                sadj = cmpp.tile([P, c.NS], F32, tag="sadj")
                nc.vector.tensor_scalar(
                    sadj[:], glf[:, :, 1], -1.0e6, 1.0e6,
                    op0=OP.mult, op1=OP.add)
                sidx_f = cmpp.tile([P, c.NS], F32, tag="sidx_f")
                nc.vector.tensor_add(sidx_f[:], sadj[:], glf[:, :, 0])
                sidx = cmpp.tile([P, c.NS], I32, tag="sidx")
                nc.vector.tensor_copy(sidx[:], sidx_f[:])
                wslot = cmpp.tile([P, c.NS], F32, tag="wslot")
                nc.vector.tensor_copy(wslot[:], glf[:, :, 2])
                if debug:
                    nc.sync.dma_start(dbg_gidx[h], gidx[:])
                    nc.sync.dma_start(dbg_sidx[h], sidx[:])
                    nc.sync.dma_start(dbg_wslot[h], wslot[:])
                    nc.sync.dma_start(dbg_pm[h], pm[:])
                    nc.sync.dma_start(dbg_idw[h], idw[:])
                    nc.sync.dma_start(dbg_glf[h], glf[:])

                # --- gather + transpose to feature-major bf16 ---
                xtg = xtgp.tile([P, c.DK, c.CAP], BF16, tag="xtg")
                for s in range(c.NS):
                    gx = xgp.tile([P, c.D], BF16, tag="gx")
                    nc.gpsimd.indirect_dma_start(
                        out=gx[:], out_offset=None, in_=x16[:],
                        in_offset=bass.IndirectOffsetOnAxis(
                            ap=gidx[:, s:s + 1], axis=0),
                    )
                    for k in range(c.DK):
                        pt = tpps.tile([P, P], BF16, tag="pt")
                        nc.tensor.transpose(
                            out=pt[:], in_=gx[:, k * P:(k + 1) * P],
                            identity=ident[:])
                        nc.vector.tensor_copy(
                            xtg[:, k, s * P:(s + 1) * P], pt[:])

                # --- expert MLP ---
                osb_tiles = {}
                for sg in range(c.NSG):
                    wg, wu, wd = [], [], []
                    for f in range(c.FCG):
                        fc = sg * c.FCG + f
                        tg = wpool.tile([P, c.DK, P], BF16, tag="wg")
                        nc.sync.dma_start(tg[:], g16[fc])
                        tu = wpool.tile([P, c.DK, P], BF16, tag="wu")
                        nc.sync.dma_start(tu[:], u16[fc])
                        td = wpool.tile([P, c.DK, P], BF16, tag="wd")
                        nc.sync.dma_start(td[:], d16[fc])
                        wg.append(tg)
                        wu.append(tu)
                        wd.append(td)

                    for t2 in range(c.NTT2):
                        po = [pops.tile([P, c.D], F32, tag="po", name=f"po{_j}")
                              for _j in range(c.JT)]
                        for f in range(c.FCG):
                            gu = gups.tile([P, 2, c.TT2], F32, tag="gu")
                            for k in range(c.DK):
                                nc.tensor.matmul(
                                    gu[:, 0, :], lhsT=wg[f][:, k, :],
                                    rhs=xtg[:, k, t2 * c.TT2:(t2 + 1) * c.TT2],
                                    start=(k == 0), stop=(k == c.DK - 1))
                            for k in range(c.DK):
                                nc.tensor.matmul(
                                    gu[:, 1, :], lhsT=wu[f][:, k, :],
                                    rhs=xtg[:, k, t2 * c.TT2:(t2 + 1) * c.TT2],
                                    start=(k == 0), stop=(k == c.DK - 1))
                            a = stgp.tile([P, c.TT2], BF16, tag="a")
                            nc.scalar.activation(a[:], gu[:, 0, :], AF.Silu)
                            nc.vector.tensor_mul(a[:], a[:], gu[:, 1, :])
                            for j in range(c.JT):
                                for nh in range(c.NH):
                                    nc.tensor.matmul(
                                        po[j][:, nh * (c.D // c.NH):(nh + 1) * (c.D // c.NH)],
                                        lhsT=a[:, j * P:(j + 1) * P],
                                        rhs=wd[f][:, nh * c.DCH:(nh + 1) * c.DCH, :]
                                            .rearrange("p a b -> p (a b)"),
                                        start=(f == 0), stop=(f == c.FCG - 1))
                        for j in range(c.JT):
                            s = t2 * c.JT + j
                            if sg == 0:
                                osb = outsbp.tile([P, c.D], F32, tag="osb")
                                osb_tiles[s] = osb
                                nc.vector.tensor_copy(osb[:], po[j][:])
                            else:
                                osb = osb_tiles[s]
                                nc.vector.tensor_add(osb[:], osb[:], po[j][:])
                            if sg == c.NSG - 1:
                                stg = stgp.tile([P, c.D], BF16, tag="sc")
                                nc.vector.tensor_scalar_mul(
                                    stg[:], osb[:], wslot[:, s:s + 1])
                                nc.gpsimd.indirect_dma_start(
                                    out=out_dram[:],
                                    out_offset=bass.IndirectOffsetOnAxis(
                                        ap=sidx[:, s:s + 1], axis=0),
                                    in_=stg[:], in_offset=None,
                                    bounds_check=c.T - 1, oob_is_err=False,
                                )

                if debug:
                    nc.sync.dma_start(dbg_xtg[h], xtg[:])
                    nc.sync.dma_start(
                        dbg_od[h * c.TBLK:(h + 1) * c.TBLK, :],
                        out_dram[h * c.TBLK:(h + 1) * c.TBLK, :])
                # --- combine: ReduceScatter this block ---
                nc.gpsimd.collective_compute(
                    "ReduceScatter", OP.add, replica_groups=rg,
                    ins=[out_dram[h * c.TBLK:(h + 1) * c.TBLK, :].opt()],
                    outs=[rs_o[h][:].opt()],
                )
                W = c.TPC // c.NBLK
                nc.gpsimd.dma_start(out[h * W:(h + 1) * W, :], rs_o[h][:])

    nc.compile()
    return nc


def _shard_inputs(cfg: Cfg, hidden_states, router_w, gate_w, up_w, down_w):
    c = cfg
    T, D, E = c.T, c.D, c.E
    x = np.ascontiguousarray(
        np.asarray(hidden_states).reshape(T, D).astype(np.float32))
    xpad = np.zeros((c.XROWS, D), np.float32)
    xpad[:T] = x
    xT = np.ascontiguousarray(x.T)  # [D, T]
    rwtv = np.ascontiguousarray(np.asarray(router_w).astype(np.float32).T)

    in_maps = []
    for e in range(NCORES):
        ge = np.asarray(gate_w[e]).astype(np.float32)   # [F, D]
        ue = np.asarray(up_w[e]).astype(np.float32)     # [F, D]
        de = np.asarray(down_w[e]).astype(np.float32)   # [D, F]
        gswv = np.ascontiguousarray(
            ge.reshape(c.NFC, P, c.DK, P).transpose(0, 3, 2, 1))
        uswv = np.ascontiguousarray(
            ue.reshape(c.NFC, P, c.DK, P).transpose(0, 3, 2, 1))
        dswv = np.ascontiguousarray(
            de.reshape(c.DK, P, c.NFC, P).transpose(2, 3, 0, 1))
        eohv = np.zeros((P, E), np.float32)
        eohv[:, e] = 1.0
        in_maps.append({
            "x": xpad,
            "xts": np.ascontiguousarray(xT[:, e * c.TPC:(e + 1) * c.TPC]),
            "rwt": rwtv,
            "gsw": gswv,
            "usw": uswv,
            "dsw": dswv,
            "eoh": eohv,
        })
    return in_maps


def _unshard(cfg: Cfg, results):
    c = cfg
    out = np.zeros((c.T, c.D), np.float32)
    W = c.TPC // c.NBLK
    for r in range(NCORES):
        o = results[r]["out"]
        for h in range(c.NBLK):
            out[h * c.TBLK + r * W: h * c.TBLK + (r + 1) * W] = \
                o[h * W:(h + 1) * W]
    logits = np.concatenate(
        [results[r]["logits"] for r in range(NCORES)], axis=0)
    return out, logits


def run_cfg(cfg, hidden_states, router_w, gate_w, up_w, down_w, trace=False):
    in_maps = _shard_inputs(cfg, hidden_states, router_w, gate_w, up_w, down_w)
    nc = build(cfg)
    res = run_bass_kernel_spmd(nc, in_maps, core_ids=list(range(NCORES)),
                               trace=False)
    out, logits = _unshard(cfg, res.results)
    return out, logits, res


def run_timed(cfg, hidden_states, router_w, gate_w, up_w, down_w, n_iter=12):
    """Run via PJRT with device-resident inputs, loop for wall timing.

    Returns (out, logits, per_iter_seconds list).
    """
    import time
    import jax
    import concourse.mybir as mb
    from jax.experimental.shard_map import shard_map
    from jax.sharding import Mesh, PartitionSpec, NamedSharding
    from concourse import bass2jax as b2j

    in_maps = _shard_inputs(cfg, hidden_states, router_w, gate_w, up_w, down_w)
    nc = build(cfg)
    b2j.install_neuronx_cc_hook()

    partition_name = (nc.partition_id_tensor.name
                      if nc.partition_id_tensor else None)
    in_names, out_names, out_avals, zero_outs = [], [], [], []
    for alloc in nc.m.functions[0].allocations:
        if not isinstance(alloc, mb.MemoryLocationSet):
            continue
        name = alloc.memorylocations[0].name
        if alloc.kind == "ExternalInput":
            if name != partition_name:
                in_names.append(name)
        elif alloc.kind == "ExternalOutput":
            shape = tuple(alloc.tensor_shape)
            dtype = mb.dt.np(alloc.dtype)
            out_names.append(name)
            out_avals.append(jax.core.ShapedArray(shape, dtype))
            zero_outs.append(np.zeros(shape, dtype))
    n_params = len(in_names)
    all_in_names = list(in_names) + list(out_names)
    if partition_name is not None:
        all_in_names.append(partition_name)

    def _body(*args):
        operands = list(args)
        if partition_name is not None:
            operands.append(b2j.partition_id_tensor())
        outs = b2j._bass_exec_p.bind(
            *operands,
            out_avals=tuple(out_avals),
            in_names=tuple(all_in_names),
            out_names=tuple(out_names),
            lowering_input_output_aliases=(),
            sim_require_finite=True,
            sim_require_nnan=True,
            nc=nc,
        )
        return tuple(outs)

    devices = jax.devices()[:NCORES]
    mesh = Mesh(np.asarray(devices), ("core",))
    n_outs = len(out_names)
    in_specs = (PartitionSpec("core"),) * (n_params + n_outs)
    out_specs = (PartitionSpec("core"),) * n_outs
    fn = jax.jit(shard_map(_body, mesh=mesh, in_specs=in_specs,
                           out_specs=out_specs, check_rep=False),
                 keep_unused=True)

    sh = NamedSharding(mesh, PartitionSpec("core"))
    concat_in = [
        jax.device_put(
            np.concatenate([np.asarray(in_maps[c][n]) for c in range(NCORES)],
                           axis=0), sh)
        for n in in_names
    ]
    concat_zeros = [
        jax.device_put(np.zeros((NCORES * z.shape[0], *z.shape[1:]), z.dtype),
                       sh)
        for z in zero_outs
    ]

    out_arrs = fn(*concat_in, *concat_zeros)
    jax.block_until_ready(out_arrs)
    times = []
    for _ in range(n_iter):
        t0 = time.perf_counter()
        out_arrs = fn(*concat_in, *concat_zeros)
        jax.block_until_ready(out_arrs)
        times.append(time.perf_counter() - t0)

    results = [
        {name: np.asarray(out_arrs[i]).reshape(NCORES, *out_avals[i].shape)[c]
         for i, name in enumerate(out_names)}
        for c in range(NCORES)
    ]
    out, logits = _unshard(cfg, results)
    return out, logits, times


def kernel(hidden_states, router_w, gate_w, up_w, down_w):
    B, S, D = hidden_states.shape
    cfg = FULL
    assert (B * S, D) == (cfg.T, cfg.D)
    out, logits, res = run_cfg(cfg, hidden_states, router_w, gate_w, up_w,
                               down_w)
    kernel.last_results = res
    return out.reshape(B, S, D), logits
